# revision 57
# baseline (speedup 1.0000x reference)
"""Trainium2 Bass kernel for the AWARE GNN message-passing network.

Data-parallel over the batch dim: 64 graphs -> 8 NeuronCores, 8 graphs/core.
Each graph's pipeline (N=512 nodes, r=256 features, 5 walk steps):
  F1 = silu(node @ Wv + bv);  Fn = F1
  per step: G = Fn@Ww;  S^T = Fn G^T (scores, kept transposed so the
  softmax over nodes-i is a free-dim softmax);  masked softmax;
  Fn <- (Fn + A@Fn) * F1;  f_T[t] = sum_i silu(Fn@Wg + bg)
  then f = normalize(concat(f_T)); 4-layer MLP -> [8, 128] per core.

v2 design notes (vs the dual-matmul baseline):
  * A@Fn is computed ONCE (natural layout); the transposed copy of
    Fn_next comes from 8 PE transposes (1536 cyc) instead of a second
    matmul set (5120 cyc).
  * silu is a single ACT op (AF.Silu); biases ride the activation's
    per-partition bias operand in transposed layouts, killing all the
    per-step PE bias matmuls and the DVE (1+tanh)*x fixups.
  * masked softmax: DVE takes the negated row-max of raw S (softmax is
    shift-invariant, so masking before the max is unnecessary); ACT Exp
    applies the bias; the 0/1 bf16 mask is applied multiplicatively
    AFTER exp, fused with the row-sum accumulation (DVE
    tensor_tensor_reduce for 2 chunks, Pool scalar_tensor_tensor for 2
    chunks - load balance).
  * 1/rowsum folds into a row-scaled ACT Copy of Fn (scale operand).
  * "+Fn" is a DVE add from PSUM; "*F1" is a Pool multiply.
  * PSUM->SBUF moves of G^T and Fn^T go over SWDGE DMA (the DMA track
    has slack; the vector engines do not).
  * MLP weights prefetched during the graph loop (w0/w2/w3 early, w1
    during L0); silu fused, everything bf16.
  * Graphs run in interleaved groups of 3 with phase-level emission so
    every in-order engine queue always has ready work.
  * walrus in this container rejects >1 sync-wait per instruction, so a
    post-pass splits extra waits onto single-wait NoOps.
"""

import numpy as np
import ml_dtypes

import concourse.bass as bass
import concourse.mybir as mybir
import concourse.tile as tile
from concourse.bass_utils import run_bass_kernel_spmd

F32 = mybir.dt.float32
FP32R = mybir.dt.float32r
BF16 = mybir.dt.bfloat16
AF = mybir.ActivationFunctionType
OP = mybir.AluOpType

N_CORES = 8
B, N, EMB, RP = 64, 512, 256, 256
GPC = B // N_CORES  # graphs per core
STEPS = 5
HID = 1536
OUT_DIM = 128


def split_multi_waits(nc, max_waits: int = 1):
    """walrus here allows only one sync-wait per instruction; split extras
    onto single-wait NoOps inserted before it on the same engine queue."""
    n_split = 0
    for f in nc.m.functions:
        for blk in f.blocks:
            new_insts = []
            for inst in blk.instructions:
                si = inst.sync_info
                waits = list(si.on_wait) if si is not None else []
                if len(waits) > max_waits:
                    extra, keep = waits[:-max_waits], waits[-max_waits:]
                    for k, w in enumerate(extra):
                        nop = mybir.InstNoOp(
                            name=f"{inst.name}-wsplit{k}",
                            sync_info=mybir.SyncInfo(on_wait=[w], on_update=[]),
                            bass_nofuse=True,
                            engine=inst.engine,
                        )
                        new_insts.append(nop)
                        n_split += 1
                    inst.sync_info = mybir.SyncInfo(
                        on_wait=keep, on_update=list(si.on_update)
                    )
                new_insts.append(inst)
            blk.instructions = new_insts
    return n_split


def build_nc(gpc=GPC):
    nc = bass.Bass()
    P = nc.declare_dram_parameter

    nodeT_d = P("nodeT", [gpc, 2, 128, N], FP32R, isOutput=False)
    # additive mask in fp8e5m2 (-57344), scaled by the 16384-valued fp8
    # ident lhsT in the S accumulation -> -9.4e8, dominating the S spread
    FP8 = mybir.dt.float8e5
    maskT_d = P("maskT", [gpc, 4, 128, N], FP8, isOutput=False)
    ident8_d = P("ident8", [128, 128], FP8, isOutput=False)
    wv_d = P("wv", [2, 128, RP], FP32R, isOutput=False)
    wvb_d = P("wvb", [128, 2], F32, isOutput=False)   # bias col-major
    ww_d = P("ww", [2, 128, RP], FP32R, isOutput=False)
    wg_d = P("wg", [2, 128, RP], FP32R, isOutput=False)
    wgb_d = P("wgb", [128, 2], F32, isOutput=False)
    w0_d = P("w0", [12, 128, HID], BF16, isOutput=False)
    w0b_d = P("w0b", [1, HID], BF16, isOutput=False)
    w1_d = P("w1", [12, 128, HID], BF16, isOutput=False)
    w1b_d = P("w1b", [1, HID], BF16, isOutput=False)
    w2_d = P("w2", [12, 128, 768], BF16, isOutput=False)
    w2b_d = P("w2b", [1, 768], BF16, isOutput=False)
    w3_d = P("w3", [6, 128, OUT_DIM], BF16, isOutput=False)
    w3b_d = P("w3b", [1, OUT_DIM], BF16, isOutput=False)
    ident_d = P("ident", [128, 128], FP32R, isOutput=False)
    ones_d = P("ones", [N], FP32R, isOutput=False)
    out_d = P("out", [gpc, OUT_DIM], F32, isOutput=True)

    with tile.TileContext(nc) as tc:
        with (
            tc.tile_pool(name="pc", bufs=1) as pc,        # persistent consts
            tc.tile_pool(name="pw", bufs=1) as pw,        # MLP weights
            tc.tile_pool(name="ptn", bufs=10) as ptn,     # [128,2,N] fp32r grp
            tc.tile_pool(name="pnat", bufs=13) as pnat,   # [128,4,RP] grp
            tc.tile_pool(name="pmsk", bufs=5) as pmsk,    # masks
            tc.tile_pool(name="ppt", bufs=4) as ppt,      # exp tiles
            tc.tile_pool(name="psc", bufs=5) as psc,      # small scalars
            tc.tile_pool(name="pmx", bufs=1) as pmx,      # mlp intermediates
            tc.tile_pool(name="pp", bufs=8, space="PSUM") as pp,
        ):
            # ---- constants / weights resident in SBUF
            def load(shape, dt_, src, tag, pool=pc, eng=nc.sync):
                t = pool.tile(shape, dt_, tag=tag, name=tag)
                eng.dma_start(out=t, in_=src)
                return t

            # startup-critical consts only; the rest load between the first
            # graphs' input DMAs (the shared DMA device serializes transfers,
            # so emission order here is the startup critical path)
            wv_sb = load([128, 2, RP], FP32R, wv_d[:].rearrange("k p r -> p k r"), "wv")
            wvb_col = load([128, 2], F32, wvb_d[:, :], "wvb")
            ident = load([128, 128], FP32R, ident_d[:, :], "ident")
            ident8 = load([128, 128], FP8, ident8_d[:, :], "ident8")

            ftall = pc.tile([128, 12, gpc], F32, tag="ftall")
            ident_bf = pc.tile([128, 128], BF16, tag="identbf")
            nc.vector.tensor_copy(out=ident_bf, in_=ident)

            # MLP weight prefetch (w0/w2/w3 now; w1 later, see below)
            def wload(wd, nk, nout, tag, k0=0):
                # chunked: the shared DMA device serves transfers in readiness
                # order, so one big prefetch would block the graph input loads
                t = pw.tile([128, nk, nout], BF16, tag=tag, name=tag)
                for kc in range(nk):
                    nc.scalar.dma_start(out=t[:, kc, :], in_=wd[k0 + kc])
                return t

            # ---- per-graph emission helpers
            def emit_ft(st, ti, g):
                """f_T[ti] accumulation: ftall[:, ti*2+rc, g] = sum_i silu(.)"""
                fnT = st["fnT"]
                for rc in range(2):
                    hp = pp.tile([128, N], F32, tag="b1", name="ftp")
                    for kc in range(2):
                        nc.tensor.matmul(
                            hp, lhsT=wg_sb[:, kc, rc * 128:(rc + 1) * 128],
                            rhs=fnT[:, kc, :], start=(kc == 0), stop=(kc == 1))
                    dump = pmx.tile([128, N], F32, tag="dump", bufs=2,
                                    name="dump")
                    nc.scalar.activation(
                        out=dump, in_=hp, func=AF.Silu,
                        bias=wgb_col[:, rc:rc + 1],
                        accum_out=ftall[:, ti * 2 + rc, g:g + 1])

            def emit_init_loads(g):
                nodeT_sb = ptn.tile([128, 2, N], FP32R, tag="tn", name="nodeT")
                nc.sync.dma_start(out=nodeT_sb, in_=nodeT_d[g].rearrange("k p i -> p k i"))
                maskT_sb = pmsk.tile([128, 4, N], FP8, tag="maskT", name="maskT")
                nc.sync.dma_start(out=maskT_sb, in_=maskT_d[g].rearrange("k p i -> p k i"))
                return nodeT_sb, maskT_sb

            def emit_init(g, nodeT_sb, maskT_sb):
                # F1^T = silu(Wv^T node^T + bv)  (bias via ACT bias operand)
                f1T = ptn.tile([128, 2, N], FP32R, tag="tn", name="f1T")
                for rc in range(2):
                    ps = pp.tile([128, N], F32, tag="b1", name="f1p")
                    for kc in range(2):
                        nc.tensor.matmul(
                            ps, lhsT=wv_sb[:, kc, rc * 128:(rc + 1) * 128],
                            rhs=nodeT_sb[:, kc, :], start=(kc == 0), stop=(kc == 1))
                    nc.scalar.activation(
                        out=f1T[:, rc, :], in_=ps, func=AF.Silu,
                        bias=wvb_col[:, rc:rc + 1])

                # F1 natural = transpose(F1^T) via PE
                f1nat = pnat.tile([128, 4, RP], FP32R, tag="nat", name="f1nat")
                for h in range(2):
                    tp = pp.tile([128, 2, RP], FP32R, tag="b1", name="f1tp")
                    for rc in range(2):
                        for it2 in range(2):
                            it = h * 2 + it2
                            nc.tensor.transpose(
                                tp[:, it2, rc * 128:(rc + 1) * 128],
                                f1T[:, rc, it * 128:(it + 1) * 128], ident)
                    nc.vector.tensor_copy(out=f1nat[:, h * 2:h * 2 + 2, :], in_=tp)

                st = {"fnT": f1T, "fnnat": f1nat, "f1nat": f1nat,
                      "mask": maskT_sb}
                emit_ft(st, 0, g)
                return st

            def emit_group_step(sts, last):
                """One walk step for the group's graphs, phase-interleaved so
                every in-order engine queue always has ready work."""
                # G^T matmuls (PE), psum -> sbuf via DVE copy
                gps = {}
                for st in sts:
                    fnT = st["fnT"]
                    gps[id(st)] = []
                    st["gt"] = ptn.tile([128, 2, N], FP32R, tag="tn", name="gt")
                    for rc in range(2):
                        gp = pp.tile([128, N], F32, tag="b1", name="gp")
                        gps[id(st)].append(gp)
                        for kc in range(2):
                            nc.tensor.matmul(
                                gp,
                                lhsT=ww_sb[:, kc, rc * 128:(rc + 1) * 128],
                                rhs=fnT[:, kc, :], start=(kc == 0), stop=(kc == 1))
                for st in sts:
                    for rc in range(2):
                        nc.scalar.activation(
                            out=st["gt"][:, rc, :], in_=gps[id(st)][rc],
                            func=AF.Copy)
                # S^T matmuls, interleaved over jt across graphs (PE)
                for st in sts:
                    st["sp"] = [None] * 4
                    st["negmax"] = psc.tile([128, 4], F32, tag="negmax", name="negmax")
                    st["rowsum"] = psc.tile([128, 4], F32, tag="rowsum", name="rowsum")
                    st["recip"] = psc.tile([128, 4], F32, tag="recip", name="recip")
                    st["pt"] = ppt.tile([128, 4, N], FP32R, tag="pt", name="pt")
                    st["fnsc"] = pnat.tile([128, 4, RP], FP32R, tag="nat", name="fnsc")
                # S matmuls: the additive -1e9 mask rides the accumulation
                # group as a bf16 ident-matmul (emitted first: its operands
                # are ready long before fnT/gt, so it fills PE idle).
                for jt in range(4):
                    for st in sts:
                        sp = pp.tile([128, N], F32, tag="b1", name=f"sp{jt}")
                        st["sp"][jt] = sp
                        fnT = st["fnT"]
                        nc.tensor.matmul(
                            sp, lhsT=ident8, rhs=st["mask"][:, jt, :],
                            start=True, stop=False)
                        for kc in range(2):
                            nc.tensor.matmul(
                                sp, lhsT=fnT[:, kc, jt * 128:(jt + 1) * 128],
                                rhs=st["gt"][:, kc, :], start=False, stop=(kc == 1))
                # masked softmax, rolled per jt so each tile's chain
                # (max -> exp(+rowsum accum) -> recip -> scaled Fn) moves ASAP
                for jt in range(4):
                    for st in sts:
                        nc.vector.tensor_reduce(
                            out=st["negmax"][:, jt:jt + 1], in_=st["sp"][jt],
                            axis=mybir.AxisListType.X, op=OP.max, negate=True)
                    for st in sts:
                        nc.scalar.activation(
                            out=st["pt"][:, jt, :], in_=st["sp"][jt], func=AF.Exp,
                            scale=1.0, bias=st["negmax"][:, jt:jt + 1],
                            accum_out=st["rowsum"][:, jt:jt + 1])
                    for st in sts:
                        nc.vector.reciprocal(
                            st["recip"][:, jt:jt + 1], st["rowsum"][:, jt:jt + 1])
                    for st in sts:
                        nc.gpsimd.tensor_scalar_mul(
                            out=st["fnsc"][:, jt, :], in0=st["fnnat"][:, jt, :],
                            scalar1=st["recip"][:, jt:jt + 1])
                # A@Fn natural (PE, jt-major so each fnsc chunk unblocks its
                # 4 matmuls immediately), +Fn (DVE), *F1 (split DVE/Pool)
                for st in sts:
                    pt, fnsc = st["pt"], st["fnsc"]
                    fnew = [pp.tile([128, 2, RP], F32, tag="b1", name=f"fnew{_h}")
                            for _h in range(2)]
                    st["fnew"] = fnew
                    for it in range(4):
                        for jt in range(4):
                            nc.tensor.matmul(
                                fnew[it // 2][:, it % 2, :],
                                lhsT=pt[:, jt, it * 128:(it + 1) * 128],
                                rhs=fnsc[:, jt, :], start=(jt == 0), stop=(jt == 3))
                for st in sts:
                    fnx = pnat.tile([128, 4, RP], F32, tag="nat", name="fnx")
                    for h in range(2):
                        nc.vector.tensor_tensor(
                            out=fnx[:, h * 2:h * 2 + 2, :], in0=st["fnew"][h],
                            in1=st["fnnat"][:, h * 2:h * 2 + 2, :], op=OP.add)
                    st["fnx"] = fnx
                for st in sts:
                    fnnat_new = pnat.tile([128, 4, RP], FP32R, tag="nat", name="fnnat")
                    nc.gpsimd.tensor_tensor(
                        out=fnnat_new, in0=st["fnx"], in1=st["f1nat"], op=OP.mult)
                    st["fnnat_next"] = fnnat_new
                # Fn^T next: 8 PE transposes + Pool copy out of PSUM
                for st in sts:
                    tps = [pp.tile([128, N], FP32R, tag="b1", name=f"tps{_h}")
                           for _h in range(2)]
                    st["tps"] = tps
                    for rc in range(2):
                        for it in range(4):
                            nc.tensor.transpose(
                                tps[rc][:, it * 128:(it + 1) * 128],
                                st["fnnat_next"][:, it, rc * 128:(rc + 1) * 128],
                                ident)
                for st in sts:
                    fnT_new = ptn.tile([128, 2, N], FP32R, tag="tn", name="fnT")
                    for rc in range(2):
                        nc.vector.tensor_copy(
                            out=fnT_new[:, rc, :], in_=st["tps"][rc])
                    st["fnT"] = fnT_new
                    st["fnnat"] = st["fnnat_next"]

            # ---- graph loop (groups interleaved for engine overlap)
            groups = [[0, 1, 2, 3], [4, 5, 6, 7]]
            assert sum(len(gr) for gr in groups) == gpc
            w_sbs = {}
            wg_sb = wgb_col = ww_sb = ones_row = ones_col = None
            ones_bf = None
            for gi, gr in enumerate(groups):
                lds = []
                for g in gr:
                    lds.append(emit_init_loads(g))
                    if gi == 0 and g == gr[0]:
                        # ft0/step-1 consts ride between g0's and g1's loads
                        wg_sb = load([128, 2, RP], FP32R,
                                     wg_d[:].rearrange("k p r -> p k r"), "wg")
                        wgb_col = load([128, 2], F32, wgb_d[:, :], "wgb")
                        ww_sb = load([128, 2, RP], FP32R,
                                     ww_d[:].rearrange("k p r -> p k r"), "ww")
                sts = []
                for g, (nsb, msb) in zip(gr, lds):
                    st = emit_init(g, nsb, msb)
                    st["g"] = g
                    sts.append(st)
                for t in range(STEPS):
                    emit_group_step(sts, t == STEPS - 1)
                    for st in sts:
                        emit_ft(st, t + 1, st["g"])
                    if gi == 0 and t == 0:
                        ones_row = load([1, N], FP32R,
                                        ones_d[:].rearrange("(o n) -> o n", o=1),
                                        "ones_row")
                        ones_col = load([128, 1], FP32R,
                                        ones_d[0:128].rearrange("(p o) -> p o", o=1),
                                        "ones_col")
                        ones_bf = pc.tile([1, 16], BF16, tag="onesbf")
                        nc.vector.tensor_copy(out=ones_bf, in_=ones_row[0:1, 0:16])
                    if gi == 0 and t == 1:  # MLP weight prefetch, mid-loop
                        w_sbs["w2"] = wload(w2_d, 12, 768, "w2")
                        w_sbs["w3"] = wload(w3_d, 6, OUT_DIM, "w3")
                    if gi == 0 and t == 2:  # second half of w0 resident
                        w_sbs["w0b"] = wload(w0_d, 6, HID, "w0b", k0=6)

            # ---- f normalization
            sq = pc.tile([128, gpc, 12], F32, tag="sq")
            for t in range(12):
                nc.vector.tensor_tensor(
                    out=sq[:, :, t], in0=ftall[:, t, :], in1=ftall[:, t, :],
                    op=OP.mult)
            essq = pc.tile([128, gpc], F32, tag="essq")
            nc.vector.tensor_reduce(
                out=essq, in_=sq, axis=mybir.AxisListType.X, op=OP.add)
            essq_r = pc.tile([128, gpc], FP32R, tag="essqr")
            nc.vector.tensor_copy(out=essq_r, in_=essq)
            n2ps = pp.tile([1, gpc], F32, tag="b1")
            nc.tensor.matmul(n2ps, lhsT=ones_col, rhs=essq_r, start=True, stop=True)
            norm_sb = pc.tile([1, gpc], F32, tag="normsb")
            nc.scalar.activation(out=norm_sb, in_=n2ps, func=AF.Sqrt)
            nc.vector.tensor_scalar_max(out=norm_sb, in0=norm_sb, scalar1=1e-12)
            recipn = pc.tile([1, gpc], F32, tag="recipn")
            nc.vector.reciprocal(recipn, norm_sb)
            recipn_r = pc.tile([1, gpc], FP32R, tag="recipnr")
            nc.vector.tensor_copy(out=recipn_r, in_=recipn)
            bcast = pp.tile([128, gpc], F32, tag="b1")
            nc.tensor.matmul(
                bcast, lhsT=ones_row[0:1, 0:128], rhs=recipn_r, start=True, stop=True)
            fnorm = pc.tile([128, 12, gpc], BF16, tag="fnorm")
            for t in range(12):
                nc.vector.tensor_tensor(
                    out=fnorm[:, t, :], in0=ftall[:, t, :], in1=bcast, op=OP.mult)

            # ---- MLP
            ones8 = ones_bf[0:1, 0:gpc]

            def mlp_layer(lhsT_at, nks, w_sb, wb_d, nout, final=False):
                wb_row = pmx.tile([1, nout], BF16, tag=f"brow{nout}{final}", name="brow")
                nc.sync.dma_start(out=wb_row, in_=wb_d[:, :])
                if nout == HID:
                    ns_sizes = [512, 512, 512]
                elif nout == 768:
                    ns_sizes = [384, 384]
                else:
                    ns_sizes = [nout]
                h_ps = [pp.tile([gpc, s], F32, tag="b1", name=f"hps{_i}")
                        for _i, s in enumerate(ns_sizes)]
                for kc in range(nks):
                    if w_sb is w0_d and kc >= 6:  # second half is resident
                        rhs_src = w_sbs["w0b"][:, kc - 6, :]
                    elif w_sb is w0_d or w_sb is w1_d:  # stream per-chunk
                        wt = pmx.tile([128, nout], BF16, tag="wchunk", bufs=3,
                                      name="wchunk")
                        nc.sync.dma_start(out=wt, in_=w_sb[kc])
                        rhs_src = wt
                    else:
                        rhs_src = w_sb[:, kc, :]
                    off = 0
                    for i, s in enumerate(ns_sizes):
                        nc.tensor.matmul(
                            h_ps[i], lhsT=lhsT_at(kc), rhs=rhs_src[:, off:off + s],
                            start=(kc == 0), stop=False)
                        off += s
                off = 0
                for i, s in enumerate(ns_sizes):
                    nc.tensor.matmul(
                        h_ps[i], lhsT=ones8, rhs=wb_row[0:1, off:off + s],
                        start=False, stop=True)
                    off += s
                if final:
                    o = pc.tile([gpc, nout], F32, tag="outsb")
                    nc.scalar.activation(out=o, in_=h_ps[0], func=AF.Copy)
                    return o
                h_sb = pmx.tile([gpc, nout], BF16, tag=f"h{nout}", name="h")
                off = 0
                for i, s in enumerate(ns_sizes):
                    nc.scalar.activation(
                        out=h_sb[0:gpc, off:off + s], in_=h_ps[i], func=AF.Silu)
                    off += s
                # transpose h -> [nout/128 chunks, gpc] for next layer's lhsT
                nkc = nout // 128
                tp = pp.tile([128, nkc, gpc], BF16, tag="b1")
                for t2 in range(nkc):
                    nc.tensor.transpose(
                        tp[:, t2, :], h_sb[0:gpc, t2 * 128:(t2 + 1) * 128],
                        ident_bf[0:gpc, 0:gpc])
                hT = pmx.tile([128, nkc, gpc], BF16, tag=f"hT{nout}", name="hT")
                nc.vector.tensor_copy(out=hT, in_=tp)
                return hT

            h0T = mlp_layer(lambda kc: fnorm[:, kc, :], 12, w0_d, w0b_d, HID)
            h1T = mlp_layer(lambda kc: h0T[:, kc, :], 12, w1_d, w1b_d, HID)
            h2T = mlp_layer(lambda kc: h1T[:, kc, :], 12, w_sbs["w2"], w2b_d, 768)
            o_sb = mlp_layer(lambda kc: h2T[:, kc, :], 6, w_sbs["w3"], w3b_d, OUT_DIM,
                             final=True)
            nc.sync.dma_start(out=out_d[:, :], in_=o_sb[0:gpc, :])

    split_multi_waits(nc)
    return nc


_NC_CACHE = {}


def _get_nc():
    if "nc" not in _NC_CACHE:
        _NC_CACHE["nc"] = build_nc()
    return _NC_CACHE["nc"]


def _prep_shared(Wv_w, Wv_b, Ww_w, Wg_w, Wg_b, W0, b0, W1, b1, W2, b2, W3, b3,
                 ident, ones):
    f32 = np.float32

    def chunks(a, p=128):
        a = np.ascontiguousarray(a, dtype=f32)
        k, n = a.shape
        return a.reshape(k // p, p, n)

    def bcol(b):  # [256] bias -> [128, 2] column-major per rc chunk
        return np.ascontiguousarray(
            np.asarray(b, dtype=f32).reshape(2, 128).T)

    return {
        "wv": chunks(Wv_w),
        "wvb": bcol(Wv_b),
        "ww": chunks(Ww_w),
        "wg": chunks(Wg_w),
        "wgb": bcol(Wg_b),
        "w0": chunks(W0).astype(ml_dtypes.bfloat16),
        "w0b": np.asarray(b0, f32).reshape(1, -1).astype(ml_dtypes.bfloat16),
        "w1": chunks(W1).astype(ml_dtypes.bfloat16),
        "w1b": np.asarray(b1, f32).reshape(1, -1).astype(ml_dtypes.bfloat16),
        "w2": chunks(W2).astype(ml_dtypes.bfloat16),
        "w2b": np.asarray(b2, f32).reshape(1, -1).astype(ml_dtypes.bfloat16),
        "w3": chunks(np.asarray(W3, dtype=f32)).astype(ml_dtypes.bfloat16),
        "w3b": np.asarray(b3, dtype=f32).reshape(1, -1).astype(ml_dtypes.bfloat16),
        "ident": ident,
        "ident8": (np.eye(128, dtype=np.float32) * 16384.0).astype(
            ml_dtypes.float8_e5m2),
        "ones": ones,
    }


def make_in_maps(inputs, gpc=GPC, n_cores=N_CORES):
    node = np.asarray(inputs["node_attribute_matrix"], dtype=np.float32)
    adj = np.asarray(inputs["adjacent_matrix"])
    shared = _prep_shared(
        np.asarray(inputs["Wv_w"]), np.asarray(inputs["Wv_b"]),
        np.asarray(inputs["Ww_w"]), np.asarray(inputs["Wg_w"]),
        np.asarray(inputs["Wg_b"]), np.asarray(inputs["W0"]),
        np.asarray(inputs["b0"]), np.asarray(inputs["W1"]),
        np.asarray(inputs["b1"]), np.asarray(inputs["W2"]),
        np.asarray(inputs["b2"]), np.asarray(inputs["W3"]),
        np.asarray(inputs["b3"]),
        np.eye(128, dtype=np.float32), np.ones(N, dtype=np.float32))

    # node^T per graph, chunked [2, 128, N]
    nodeT = np.ascontiguousarray(node.transpose(0, 2, 1)).reshape(B, 2, 128, N)
    # additive mask, transposed: maskT[g, j, i] = 0 if adj[g,i,j] else
    # -57344 (fp8e5m2), scaled on-device by the 16384-valued ident lhsT to
    # -9.4e8 -- it must dominate the full S spread (~2.2e8 by step 5).
    adjT = adj.transpose(0, 2, 1)
    maskT = np.where(adjT != 0, np.float32(0.0), np.float32(-57344.0))
    maskT = maskT.reshape(B, 4, 128, N).astype(ml_dtypes.float8_e5m2)

    in_maps = []
    for c in range(n_cores):
        g0 = c * gpc
        m = dict(shared)
        m["nodeT"] = np.ascontiguousarray(nodeT[g0:g0 + gpc])
        m["maskT"] = np.ascontiguousarray(maskT[g0:g0 + gpc])
        in_maps.append(m)
    return in_maps


def kernel(**inputs):
    nc = _get_nc()
    in_maps = make_in_maps(inputs)
    res = run_bass_kernel_spmd(nc, in_maps, core_ids=list(range(N_CORES)))
    return np.concatenate([r["out"] for r in res.results], axis=0)
